# revision 2
# baseline (speedup 1.0000x reference)
"""FViTBackbone kernel for 8 Trainium2 NeuronCores.

Strategy: the final 3x3 conv (768->768 at 128x128, ~696 of ~805 GFLOP) runs
on the 8 NeuronCores as a Bass/Tile matmul kernel (bf16 operands, fp32 PSUM
accumulation), sharded over (image, half-height): core c handles rows
[64*(c%2), 64*(c%2)+64) of image c//2. The small stage pipeline (patch embed,
6 factorized-attention blocks, downsamples, inv branches, tconv_full + inorm)
is computed in float32 numpy on the host inside kernel().
"""

import sys

sys.path.insert(0, "/opt/trn_rl_repo")

import numpy as np
import ml_dtypes

EPS = 1e-5
PATCH = 4
HEADS = [4, 8, 12]


# ---------------- numpy reference-equivalent stage pipeline ----------------

def _inorm(x):
    m = x.mean(axis=(2, 3), keepdims=True)
    v = x.var(axis=(2, 3), keepdims=True)
    return (x - m) / np.sqrt(v + EPS)


def _relu(x):
    return np.maximum(x, 0.0)


def _mha(x, w_in, b_in, w_out, b_out, heads):
    B, S, C = x.shape
    d = C // heads
    qkv = x @ w_in.T + b_in
    q, k, v = qkv[..., :C], qkv[..., C:2 * C], qkv[..., 2 * C:]
    q = q.reshape(B, S, heads, d) * (d ** -0.5)
    k = k.reshape(B, S, heads, d)
    v = v.reshape(B, S, heads, d)
    a = np.matmul(q.transpose(0, 2, 1, 3), k.transpose(0, 2, 3, 1))  # B,h,S,S
    a = a - a.max(axis=-1, keepdims=True)
    np.exp(a, out=a)
    a /= a.sum(axis=-1, keepdims=True)
    ctx = np.matmul(a, v.transpose(0, 2, 1, 3))  # B,h,S,d
    ctx = ctx.transpose(0, 2, 1, 3).reshape(B, S, C)
    return ctx @ w_out.T + b_out


def _factor_attn(x, p, heads):
    N, C, H, W = x.shape
    t = np.ascontiguousarray(x.transpose(0, 2, 3, 1))  # N,H,W,C
    v = _mha(t.reshape(N * H, W, C), p["v_win"], p["v_bin"],
             p["v_wout"], p["v_bout"], heads).reshape(N, H, W, C)
    th = np.ascontiguousarray(t.transpose(0, 2, 1, 3)).reshape(N * W, H, C)
    h = _mha(th, p["h_win"], p["h_bin"], p["h_wout"], p["h_bout"],
             heads).reshape(N, W, H, C).transpose(0, 2, 1, 3)
    return (v + h).transpose(0, 3, 1, 2)


def _grouped_mlp(x, w1, b1, w2, b2):
    h = _relu(x[:, :, None] * w1[None, :, :, None, None]
              + b1[None, :, :, None, None])
    return (h * w2[None, :, :, None, None]).sum(axis=2) + b2[None, :, None, None]


def _block(x, p, heads):
    x = _inorm(x)
    x = _factor_attn(x, p, heads) + x
    x = _inorm(x)
    x = _grouped_mlp(x, p["mlp_w1"], p["mlp_b1"], p["mlp_w2"], p["mlp_b2"]) + x
    return x


def _conv3x3_s2(x, w, b):
    N, C, H, W = x.shape
    O = w.shape[0]
    Ho, Wo = H // 2, W // 2
    xp = np.pad(x, ((0, 0), (0, 0), (1, 1), (1, 1)))
    out = np.zeros((N, O, Ho, Wo), np.float32)
    for dy in range(3):
        for dx in range(3):
            xs = xp[:, :, dy:dy + H:2, dx:dx + W:2]
            out += np.tensordot(xs, w[:, :, dy, dx],
                                axes=([1], [1])).transpose(0, 3, 1, 2)
    return out + b[None, :, None, None]


def _tconv_dw(x, w, b):
    N, C, H, W = x.shape
    k = w.shape[-1]
    y = (x[:, :, :, None, :, None] * w[None, :, None, :, None, :]
         ).reshape(N, C, H * k, W * k)
    return y + b[None, :, None, None]


def _to_np(t):
    return np.asarray(t, dtype=np.float32)


def _tree_np(d):
    if isinstance(d, dict):
        return {k: _tree_np(v) for k, v in d.items()}
    if isinstance(d, list):
        return [_tree_np(v) for v in d]
    return _to_np(d)


def _host_stages(images, params):
    """Everything up to (and including) relu(inorm(tconv_full(concat)))."""
    pe = params["patch_embed"]
    w = pe["w"].reshape(96, 48)
    x = images.reshape(4, 3, 64, 4, 64, 4).transpose(0, 2, 4, 1, 3, 5)
    x = x.reshape(4, 64, 64, 48) @ w.T + pe["b"]
    x = np.ascontiguousarray(x.transpose(0, 3, 1, 2))  # 4,96,64,64
    x = _relu(_inorm(x))

    feats = []
    for s, (stage, ie) in enumerate(zip(params["stages"], params["inv"])):
        for blk in stage["blocks"]:
            x = _block(x, blk, HEADS[s])
        x = _conv3x3_s2(x, stage["down_w"], stage["down_b"])
        y = _tconv_dw(x, ie["tconv_w"], ie["tconv_b"])
        y = _relu(_inorm(y))
        y = np.tensordot(y, ie["proj_w"], axes=([1], [1])).transpose(0, 3, 1, 2)
        y = y + ie["proj_b"][None, :, None, None]
        y = _relu(_inorm(y))
        feats.append(y)
    out = np.concatenate(feats, axis=1)  # 4,768,64,64

    o = params["out"]
    # tconv_full: y[n,o,2h+a,2w+b] = sum_c x[n,c,h,w] * w[c,o,a,b] + bias
    tw = o["tconv_w"]  # (768,768,2,2)
    y = np.tensordot(out, tw, axes=([1], [0]))        # n,h,w,o,a,b
    y = y.transpose(0, 3, 1, 4, 2, 5).reshape(4, 768, 128, 128)
    y = y + o["tconv_b"][None, :, None, None]
    y = _relu(_inorm(y))
    return np.ascontiguousarray(y)  # 4,768,128,128 fp32


# ---------------- Bass kernel: final 3x3 conv on 8 cores ----------------

_BF16 = ml_dtypes.bfloat16


def _build_conv_bass():
    import concourse.bass as bass
    from concourse import bacc
    import concourse.mybir as mybir
    from concourse.tile import TileContext

    nc = bacc.Bacc("TRN2", target_bir_lowering=False, debug=False)
    xin = nc.dram_tensor("xpad", [128, 6 * 66 * 130], mybir.dt.bfloat16,
                         kind="ExternalInput")
    w2 = nc.dram_tensor("w2", [128, 54 * 768], mybir.dt.bfloat16,
                        kind="ExternalInput")
    outd = nc.dram_tensor("out", [128, 6, 64, 128], mybir.dt.float32,
                          kind="ExternalOutput")

    with TileContext(nc) as tc:
        with (
            tc.tile_pool(name="wx", bufs=1) as wx,
            tc.tile_pool(name="ot", bufs=4) as otp,
            tc.tile_pool(name="ps", bufs=8, space="PSUM") as psp,
        ):
            xt = wx.tile([128, 6, 66, 130], mybir.dt.bfloat16, tag="x")
            wt = wx.tile([128, 54, 768], mybir.dt.bfloat16, tag="w")
            nc.sync.dma_start(xt[:].rearrange("p a b c -> p (a b c)"), xin[:])
            nc.sync.dma_start(wt[:].rearrange("p a b -> p (a b)"), w2[:])

            for b in range(16):           # bands of 4 output rows
                for mt in range(6):       # output-channel tiles of 128
                    pt = psp.tile([128, 512], mybir.dt.float32, tag="ps")
                    first = True
                    for tap in range(9):
                        dy, dx = tap // 3, tap % 3
                        for ks in range(6):
                            nc.tensor.matmul(
                                pt[:],
                                lhsT=wt[:, tap * 6 + ks,
                                        mt * 128:(mt + 1) * 128],
                                rhs=xt[:, ks, 4 * b + dy:4 * b + dy + 4,
                                       dx:dx + 128],
                                start=first,
                                stop=(tap == 8 and ks == 5),
                            )
                            first = False
                    ot = otp.tile([128, 512], mybir.dt.float32, tag="ot")
                    nc.any.tensor_copy(out=ot[:], in_=pt[:])
                    nc.sync.dma_start(
                        outd[:, mt, 4 * b:4 * b + 4, :]
                        .rearrange("p r c -> p (r c)"),
                        ot[:],
                    )
    nc.compile()
    return nc


def kernel(images, params):
    from concourse.bass_utils import run_bass_kernel_spmd

    images = _to_np(images)
    params = _tree_np(params)

    y = _host_stages(images, params)          # (4,768,128,128) fp32
    o = params["out"]
    cw = o["conv_w"]                          # (768,768,3,3)

    # weight layout: w2l[p, (dy*3+dx)*6+ks, m] = cw[m, ks*128+p, dy, dx]
    w2l = cw.transpose(1, 2, 3, 0).reshape(6, 128, 3, 3, 768)
    w2l = w2l.transpose(1, 2, 3, 0, 4).reshape(128, 54, 768)
    w2l = np.ascontiguousarray(w2l).astype(_BF16)

    in_maps = []
    for c in range(8):
        n, half = c // 2, c % 2
        r0 = 64 * half
        pad = np.zeros((768, 66, 130), np.float32)
        lo = max(r0 - 1, 0)
        hi = min(r0 + 65, 128)
        pad[:, lo - (r0 - 1):hi - (r0 - 1), 1:129] = y[n, :, lo:hi, :]
        xp = pad.reshape(6, 128, 66, 130).transpose(1, 0, 2, 3)
        xp = np.ascontiguousarray(xp).reshape(128, 6 * 66 * 130).astype(_BF16)
        in_maps.append({"xpad": xp, "w2": w2l})

    nc = _build_conv_bass()
    res = run_bass_kernel_spmd(nc, in_maps, list(range(8)))

    out = np.empty((4, 768, 128, 128), np.float32)
    for c in range(8):
        n, half = c // 2, c % 2
        r0 = 64 * half
        blk = res.results[c]["out"]           # (128, 6, 64, 128)
        out[n, :, r0:r0 + 64, :] = blk.transpose(1, 0, 2, 3).reshape(768, 64, 128)
    out += o["conv_b"][None, :, None, None]
    return out
